# revision 39
# baseline (speedup 1.0000x reference)
# GAT (graph attention) layer on 8 Trainium2 NeuronCores.
#
# Strategy: target-sharded edges. Each core owns 1/8 of the target nodes and
# processes exactly the edges pointing into its range, so the softmax
# denominator and the weighted-feature aggregation are core-local matmuls.
# There is NO collective: the reference's global-max exp shift cancels
# exactly between numerator and denominator (both carry exp(-24) here), and
# its +1e-16 epsilon is replaced by max(denom, 1e-30), which only differs on
# zero-in-degree targets (where both give att=0).
#
# Per core:
#   Phase T1: proj|s_src for ALL nodes from host-transposed xT via one matmul
#     per 128-node tile -> bf16 gather table rows [proj(128)|ss_hi(4)|ss_lo(4)]
#     at 512B stride in DRAM.
#   Phase T2: local s_trg table (hi/lo bf16) into a persistent SBUF tile.
#   Phase E (fused E+F), per window of 128 target nodes: two dma_gathers fetch
#     per-edge source rows; st per edge via one-hot fp8 matmul against the
#     local s_trg window column; z = ss + st, ex = exp(leaky(z) - 24);
#     sel one-hot built on-chip (is_equal vs iota); ONE accumulating matmul
#     per edge tile with lhsT=sel, rhs=[proj*ex | ex] yields
#     [target, features|denoms] directly; divide, add skip(x+bias), ELU, out.
import sys
from contextlib import ExitStack

import numpy as np

sys.path.insert(0, "/opt/trn_rl_repo")

import ml_dtypes  # noqa: E402

import concourse.bass as bass  # noqa: E402,F401
import concourse.bass_isa as bass_isa  # noqa: E402
import concourse.mybir as mybir  # noqa: E402
import concourse.tile as tile  # noqa: E402
from concourse import bacc  # noqa: E402

P = 128
NH, FOUT = 4, 32
NHF = NH * FOUT  # 128
FIN = 128
ROW = 2 * P  # table row: 256 bf16 = 512B (136 used)
LEAKY = 0.2
SHIFT = 24.0
N_CORES = 8
F32 = mybir.dt.float32
F16 = mybir.dt.float16
BF16 = mybir.dt.bfloat16
FP8 = mybir.dt.float8e4
I16 = mybir.dt.int16
I32 = mybir.dt.int32
OP = mybir.AluOpType
ACT = mybir.ActivationFunctionType
BF = ml_dtypes.bfloat16
F8 = ml_dtypes.float8_e4m3
GT = 3  # node tiles per phase-T group (PSUM: 3*136 f32 <= 512)


def _wrap16(flat):
    """[..., L] -> dma_gather layout [..., 16, L//16] replicated to 128 rows."""
    L = flat.shape[-1]
    w = flat.reshape(flat.shape[:-1] + (L // 16, 16))
    w = np.swapaxes(w, -1, -2)  # [..., 16, L//16]
    return np.tile(w, (1, 1, 8, 1)).reshape(flat.shape[:-1] + (P, L // 16))


def _prepare_edges(edge_index, n_nodes, n_cores, half, padskip=False):
    npc = n_nodes // n_cores  # 6250
    nw = (npc + P - 1) // P  # 49
    src = np.ascontiguousarray(edge_index[0]).astype(np.int64)
    trg = np.ascontiguousarray(edge_index[1]).astype(np.int64)
    E = src.shape[0]
    wglob = (trg // npc) * nw + (trg % npc) // P
    isb = (src >= half).astype(np.int64)
    order = np.argsort(wglob * 2 + isb, kind="stable")
    src_s, trg_s, wg_s, isb_s = src[order], trg[order], wglob[order], isb[order]
    nwin = n_cores * nw
    cnt = np.bincount(wg_s * 2 + isb_s, minlength=2 * nwin)
    t_a = max(1, int(np.ceil(cnt[0::2].max() / P)))
    t_b = max(1, int(np.ceil(cnt[1::2].max() / P)))
    t_eff = t_a + t_b
    gkey = wg_s * 2 + isb_s
    gstart = np.concatenate([[0], np.cumsum(np.bincount(gkey, minlength=2 * nwin))])[:-1]
    jj = np.arange(E) - gstart[gkey]
    t_loc = jj // P
    p_idx = jj % P
    t_idx = np.where(isb_s == 1, t_a + t_loc, t_loc)
    c = wg_s // nw
    wloc = wg_s % nw
    rel = (trg_s % npc) - wloc * P  # 0..127

    fill = -1 if padskip else 0  # -1: trailing pads are skipped by dma_gather
    idx_a = np.full((n_cores, nw, t_a * P), fill, np.int16)
    idx_b = np.full((n_cores, nw, t_b * P), fill, np.int16)
    ma = isb_s == 0
    idx_a[c[ma], wloc[ma], t_loc[ma] * P + p_idx[ma]] = src_s[ma].astype(np.int16)
    mb = isb_s == 1
    idx_b[c[mb], wloc[mb], t_loc[mb] * P + p_idx[mb]] = (src_s[mb] - half).astype(np.int16)

    rel_arr = np.full((n_cores, nw * P, t_eff), -1.0, np.float32)
    rel_arr[c, wloc * P + p_idx, t_idx] = rel
    selt = np.zeros((n_cores, nw * P, t_eff * P), F8)
    selt[c, wloc * P + rel, t_idx * P + p_idx] = 1.0

    ia = _wrap16(idx_a)  # [nc, nw, 128, t_a*8]
    ib = _wrap16(idx_b)
    gidx = np.concatenate([ia, ib], axis=-1).reshape(n_cores, nw * P, t_eff * 8)
    return t_a, t_b, gidx, rel_arr.astype(BF), selt


def build_bass(n_nodes, n_cores, t_a, t_b, mock_cc=False, phases="full",
               sp=False, qb=0, tail="pe", ngather=False, nselt=False,
               padskip=False):
    npc = n_nodes // n_cores  # 6250
    nw = (npc + P - 1) // P  # 49
    npad = nw * P  # 6272
    ntp = 392  # node tiles incl padding (130 groups of 3 + 1 group of 2)
    n_tab = ntp * P  # 50176
    half = n_tab // 2  # 25088
    t_eff = t_a + t_b
    nc = bacc.Bacc("TRN2", target_bir_lowering=False, debug=False,
                   num_devices=n_cores)

    xT = nc.dram_tensor("xT", [FIN, n_tab], F16, kind="ExternalInput")
    xlocT = nc.dram_tensor("xlocT", [FIN, npad], F16, kind="ExternalInput")
    wcat_in = nc.dram_tensor("wcat", [FIN, NHF + 2 * NH], F16,
                             kind="ExternalInput")
    if phases != "T":
        gidx = nc.dram_tensor("gidx", [nw * P, t_eff * 8], I16,
                              kind="ExternalInput")
        trg_rel = nc.dram_tensor("trg_rel", [nw * P, t_eff], BF16,
                                 kind="ExternalInput")
        selt_in = nc.dram_tensor("selt", [nw * P, t_eff * P], FP8,
                                 kind="ExternalInput")
    if phases == "full":
        xbloc = nc.dram_tensor("xbloc", [npad, NHF], F32, kind="ExternalInput")
    out = nc.dram_tensor("out", [npad, NHF], F32, kind="ExternalOutput")

    tab = nc.dram_tensor("tab", [n_tab, ROW], BF16)

    with tile.TileContext(nc) as tc, ExitStack() as ctx:
        const = ctx.enter_context(tc.tile_pool(name="const", bufs=1))

        c_i32 = const.tile([P, P], I32)
        nc.gpsimd.iota(c_i32[:], pattern=[[1, P]], base=0, channel_multiplier=0)
        c_bf = const.tile([P, P], BF16)
        nc.vector.tensor_copy(c_bf[:], c_i32[:])

        wcat = const.tile([FIN, NHF + 2 * NH], F16)
        nc.sync.dma_start(wcat[:], wcat_in[:])
        neg24 = const.tile([P, 1], F32)
        nc.gpsimd.memset(neg24[:], -SHIFT)
        zcol = const.tile([P, 1], F32)
        nc.gpsimd.memset(zcol[:], 0.0)
        tabc = const.tile([P, nw * 2 * NH], BF16)  # local s_trg hi/lo
        tabc3 = tabc[:].rearrange("p (w c) -> p w c", c=2 * NH)
        zlmax = const.tile([P, (t_a + t_b) * NH], F32)
        nc.gpsimd.memset(zlmax[:], -1e30)
        negbig = const.tile([P, 1], F32)
        nc.gpsimd.memset(negbig[:], -1e30)
        zerob = const.tile([P, 1], BF16)
        nc.gpsimd.memset(zerob[:], 0.0)
        num_sb = const.tile([P, nw * (NHF + NH)], F32)  # per-window [feat|den]
        num3 = num_sb[:].rearrange("p (w c) -> p w c", c=NHF + NH)

        # one-shot preloads of all per-window metadata (SBUF-resident)
        if phases != "T":
            gidx_sb = const.tile([P, nw * t_eff * 8], I16)
            gidx3 = gidx_sb[:].rearrange("p (w c) -> p w c", c=t_eff * 8)
            nc.sync.dma_start(
                gidx3, gidx[:].rearrange("(w p) c -> p w c", p=P))
            rel_sb = const.tile([P, nw * t_eff], BF16)
            rel3 = rel_sb[:].rearrange("p (w c) -> p w c", c=t_eff)
            nc.sync.dma_start(
                rel3, trg_rel[:].rearrange("(w p) c -> p w c", p=P))
        if phases == "full":
            xb_sb = const.tile([P, nw * NHF], F32)
            xb3 = xb_sb[:].rearrange("p (w c) -> p w c", c=NHF)
            nc.sync.dma_start(
                xb3, xbloc[:].rearrange("(w p) c -> p w c", p=P))

        # --- phase T1: global gather table [proj | ss_hi | ss_lo] ---
        with tc.tile_pool(name="sbT", bufs=3) as sbT, \
             tc.tile_pool(name="psT", bufs=2, space="PSUM") as psT:
            g0 = 0
            gi = 0
            while g0 < ntp:
                tg = min(GT, ntp - g0)
                cols = tg * P
                xt = sbT.tile([P, GT * P], F16, tag="xt")
                nc.sync.dma_start(xt[:, 0:cols], xT[:, g0 * P:g0 * P + cols])
                ps = psT.tile([P, GT * (NHF + NH)], F32, tag="ps")
                ps3 = ps[:].rearrange("p (t c) -> p t c", c=NHF + NH)
                for t in range(tg):
                    nc.tensor.matmul(ps3[:, t, :],
                                     lhsT=xt[:, t * P:(t + 1) * P],
                                     rhs=wcat[:, 0:NHF + NH],
                                     start=True, stop=True)
                tabt = sbT.tile([P, GT * ROW], BF16, tag="tabt")
                t3 = tabt[:].rearrange("p (t c) -> p t c", c=ROW)
                t4 = tabt[:].rearrange("p (t d c) -> p t d c", d=2, c=P)
                # proj into cols 0:128 AND (stale copy) 128:256 so the full
                # 512B row is initialized; ss_hi then overwrites 128:132
                nc.scalar.activation(
                    t4[:, 0:tg, :, :],
                    ps3[:, 0:tg, None, 0:NHF].to_broadcast([P, tg, 2, NHF]),
                    ACT.Copy)
                nc.scalar.activation(t3[:, 0:tg, NHF:NHF + NH],
                                     ps3[:, 0:tg, NHF:NHF + NH], ACT.Copy)
                nc.vector.tensor_tensor(t3[:, 0:tg, NHF + NH:NHF + 2 * NH],
                                        ps3[:, 0:tg, NHF:NHF + NH],
                                        t3[:, 0:tg, NHF:NHF + NH], OP.subtract)
                od = tab[g0 * P:g0 * P + cols, :]
                od3 = od.rearrange("(t p) c -> p t c", p=P)
                nc.sync.dma_start(od3, t3[:, 0:tg, :])
                g0 += tg

            # --- phase T2: local s_trg hi/lo into SBUF (49 tiles) ---
            g0 = 0
            while g0 < nw:
                tg = min(GT, nw - g0)
                cols = tg * P
                xt = sbT.tile([P, GT * P], F16, tag="xt")
                nc.sync.dma_start(xt[:, 0:cols], xlocT[:, g0 * P:g0 * P + cols])
                ps = psT.tile([P, GT * (NHF + NH)], F32, tag="ps")
                ps3 = ps[:].rearrange("p (t c) -> p t c", c=NHF + NH)
                for t in range(tg):
                    nc.tensor.matmul(ps3[:, t, 0:NH],
                                     lhsT=xt[:, t * P:(t + 1) * P],
                                     rhs=wcat[:, NHF + NH:NHF + 2 * NH],
                                     start=True, stop=True)
                nc.scalar.activation(tabc3[:, g0:g0 + tg, 0:NH],
                                     ps3[:, 0:tg, 0:NH], ACT.Copy)
                nc.vector.tensor_tensor(tabc3[:, g0:g0 + tg, NH:2 * NH],
                                        ps3[:, 0:tg, 0:NH],
                                        tabc3[:, g0:g0 + tg, 0:NH], OP.subtract)
                g0 += tg

        # --- phase E: per target window, fused edge + output stage ---
        sb = ctx.enter_context(tc.tile_pool(name="sb", bufs=3))
        sbg = ctx.enter_context(tc.tile_pool(name="sbg", bufs=2))
        psE = ctx.enter_context(tc.tile_pool(name="psE", bufs=2, space="PSUM"))

        if phases == "T":  # timing probe: stop after table build
            dummy = sb.tile([P, NHF], F32, tag="dummy")
            nc.vector.tensor_copy(dummy[:], wcat[:, 0:NHF])
            nc.sync.dma_start(out[0:P, :], dummy[:])

        for w in range(nw if phases != "T" else 0):
            er0 = w * P
            selt = sbg.tile([P, t_eff * P], FP8, tag="selt")
            if nselt:  # timing probe: selt DMA replaced by DVE memset
                nc.vector.memset(selt[:], 1.0)
            else:
                nc.sync.dma_start(selt[:], selt_in[er0:er0 + P, :])

            gath = sbg.tile([P, t_eff * ROW], BF16, tag="gath")
            g3 = gath[:].rearrange("p (t c) -> p t c", c=ROW)
            if ngather:  # timing probe: gathers replaced by DVE memset
                nc.vector.memset(gath[:], 0.0)
            else:
                nc.gpsimd.dma_gather(
                    out_ap=g3[:, 0:t_a, :], in_ap=tab[0:half, :],
                    idxs_ap=gidx3[:, w, 0:t_a * 8], num_idxs=t_a * P,
                    num_idxs_reg=t_a * P, elem_size=ROW, single_packet=sp)
                nc.gpsimd.dma_gather(
                    out_ap=g3[:, t_a:t_eff, :], in_ap=tab[half:n_tab, :],
                    idxs_ap=gidx3[:, w, t_a * 8:t_eff * 8], num_idxs=t_b * P,
                    num_idxs_reg=t_b * P, elem_size=ROW, single_packet=sp,
                    queue_num=qb)

            # st per edge: one-hot fp8 select matmul against local window col
            ps_st = psE.tile([P, t_eff * 2 * NH], F32, tag="ps_st")
            st3 = ps_st[:].rearrange("p (t h) -> p t h", h=2 * NH)
            for t in range(t_eff):
                nc.tensor.matmul(st3[:, t, :],
                                 lhsT=selt[:, t * P:(t + 1) * P],
                                 rhs=tabc3[:, w, :], start=True, stop=True)

            # z = ss_hi + ss_lo + st_hi + st_lo ; ex = exp(leaky(z) - 24)
            z = sb.tile([P, t_eff * NH], F32, tag="z")
            z3 = z[:].rearrange("p (t h) -> p t h", h=NH)
            nc.vector.tensor_tensor(z3, g3[:, :, NHF:NHF + NH],
                                    g3[:, :, NHF + NH:NHF + 2 * NH], OP.add)
            nc.vector.tensor_tensor(z3, z3, st3[:, :, 0:NH], OP.add)
            nc.vector.tensor_tensor(z3, z3, st3[:, :, NH:2 * NH], OP.add)
            zl = sb.tile([P, t_eff * NH], F32, tag="zl")
            nc.vector.tensor_scalar_mul(zl[:], z[:], LEAKY)
            nc.vector.tensor_tensor(zl[:], zl[:], z[:], OP.max)
            if padskip:
                # pad slots hold stale gather data; force them to -1e30 so
                # neither zlmax nor ex sees garbage
                padmask = sb.tile([P, t_eff], BF16, tag="padmask")
                nc.vector.tensor_scalar(padmask[:], rel3[:, w, :], -1.0, None,
                                        OP.is_equal)
                zl3 = zl[:].rearrange("p (t h) -> p t h", h=NH)
                nc.vector.copy_predicated(
                    zl3, padmask[:, :, None].to_broadcast([P, t_eff, NH]),
                    negbig[:, 0:1, None].to_broadcast([P, t_eff, NH]))
            nc.vector.tensor_tensor(zlmax[:], zlmax[:], zl[:], OP.max)
            ex = sb.tile([P, t_eff * NH], BF16, tag="ex")
            nc.scalar.activation(ex[:], zl[:], ACT.Exp, bias=neg24[:])
            ex3 = ex[:].rearrange("p (t h) -> p t h", h=NH)

            # sel one-hot (edge-partition layout), built on-chip
            sel = sbg.tile([P, t_eff * P], BF16, tag="sel")
            relw = rel3[:, w, :]
            nc.vector.tensor_tensor(
                sel[:].rearrange("p (t q) -> p t q", q=P),
                relw[:, :, None].to_broadcast([P, t_eff, P]),
                c_bf[:, None, :].to_broadcast([P, t_eff, P]),
                OP.is_equal)

            # wgt = [proj * ex | ex]
            wgt = sbg.tile([P, t_eff * (NHF + NH)], BF16, tag="wgt")
            w3 = wgt[:].rearrange("p (t c) -> p t c", c=NHF + NH)
            nc.vector.tensor_tensor(
                w3[:, :, 0:NHF].rearrange("p t (h f) -> p t h f", f=FOUT),
                g3[:, :, 0:NHF].rearrange("p t (h f) -> p t h f", f=FOUT),
                ex3[:, :, :, None].to_broadcast([P, t_eff, NH, FOUT]),
                OP.mult)
            nc.vector.tensor_copy(w3[:, :, NHF:NHF + NH], ex3)
            if padskip:
                # stale-garbage proj x ex can be NaN; force pad slots to 0
                nc.vector.copy_predicated(
                    w3, padmask[:, :, None].to_broadcast([P, t_eff, NHF + NH]),
                    zerob[:, 0:1, None].to_broadcast([P, t_eff, NHF + NH]))

            # accumulate [targets, feats | denom] in one PSUM tile
            ps_o = psE.tile([P, NHF + NH], F32, tag="ps_o")
            for t in range(t_eff):
                nc.tensor.matmul(ps_o[:], lhsT=sel[:, t * P:(t + 1) * P],
                                 rhs=wgt[:, t * (NHF + NH):(t + 1) * (NHF + NH)],
                                 start=(t == 0), stop=(t == t_eff - 1))
            nc.vector.tensor_copy(num3[:, w, :], ps_o[:])

        if phases in ("TE", "TEC"):  # timing probes
            o0 = sb.tile([P, NHF], F32, tag="o0")
            nc.vector.tensor_copy(o0[:], num3[:, 0, 0:NHF])
            nc.sync.dma_start(out[0:P, :], o0[:])

        if phases in ("full", "TEC"):
            # --- global max M (tiny AllReduce) -> ceps = 1e-16*exp(M-24) ---
            zm1 = sb.tile([P, 1], F32, tag="zm1")
            nc.vector.tensor_reduce(zm1[:], zlmax[:],
                                    axis=mybir.AxisListType.X, op=OP.max)
            dram = ctx.enter_context(tc.tile_pool(name="dram", bufs=1,
                                                  space="DRAM"))
            psC = ctx.enter_context(tc.tile_pool(name="psC", bufs=1,
                                                 space="PSUM"))
            if tail == "pe":
                # cross-partition max without gpsimd: PE transpose + DVE
                from concourse.masks import make_identity
                ident = const.tile([P, P], F32)
                make_identity(nc, ident[:])
                ps_zr = psC.tile([1, P], F32, tag="ps_zr")
                nc.tensor.matmul(ps_zr[:], lhsT=zm1[:], rhs=ident[:],
                                 start=True, stop=True)
                zrow = sb.tile([1, P], F32, tag="zrow")
                nc.vector.tensor_copy(zrow[:], ps_zr[:])
                pm = sb.tile([1, 1], F32, tag="pm1")
                nc.vector.tensor_reduce(pm[:], zrow[:],
                                        axis=mybir.AxisListType.X, op=OP.max)
                cc_in = dram.tile([1, 1], F32)
                cc_out = dram.tile([1, 1], F32)
                nc.sync.dma_start(cc_in[:], pm[:])
                if mock_cc:
                    mglob = nc.dram_tensor("mglob", [1, 1], F32,
                                           kind="ExternalInput")
                    nc.sync.dma_start(cc_out[:], mglob[:])
                else:
                    nc.gpsimd.collective_compute(
                        "AllReduce", OP.max,
                        replica_groups=[list(range(n_cores))],
                        ins=[cc_in.opt()], outs=[cc_out.opt()])
                cs = sb.tile([1, 1], F32, tag="cs")
                nc.sync.dma_start(cs[:], cc_out[:])
                nc.scalar.activation(cs[:], cs[:], ACT.Exp, bias=neg24[:1])
                nc.vector.tensor_scalar_mul(cs[:], cs[:], 1e-16)
                ones_col = const.tile([1, P], F32)
                nc.gpsimd.memset(ones_col[:], 1.0)
                ps_cb = psC.tile([P, 1], F32, tag="ps_cb")
                nc.tensor.matmul(ps_cb[:], lhsT=ones_col[:], rhs=cs[:],
                                 start=True, stop=True)
                ceps = const.tile([P, 1], F32)
                nc.vector.tensor_copy(ceps[:], ps_cb[:])
            else:
                pm = sb.tile([P, 1], F32, tag="pm")
                nc.gpsimd.partition_all_reduce(pm[:], zm1[:], channels=P,
                                               reduce_op=bass_isa.ReduceOp.max)
                cc_in = dram.tile([P, 1], F32)
                cc_out = dram.tile([P, 1], F32)
                nc.sync.dma_start(cc_in[:], pm[:])
                if mock_cc:
                    mglob = nc.dram_tensor("mglob", [P, 1], F32,
                                           kind="ExternalInput")
                    nc.sync.dma_start(cc_out[:], mglob[:])
                else:
                    nc.gpsimd.collective_compute(
                        "AllReduce", OP.max,
                        replica_groups=[list(range(n_cores))],
                        ins=[cc_in.opt()], outs=[cc_out.opt()])
                ceps = const.tile([P, 1], F32)
                nc.sync.dma_start(ceps[:], cc_out[:])
                nc.scalar.activation(ceps[:], ceps[:], ACT.Exp, bias=neg24[:])
                nc.vector.tensor_scalar_mul(ceps[:], ceps[:], 1e-16)

        # --- phase F2: divide (+eps), skip+bias, ELU ---
        for w in range(nw if phases == "full" else 0):
            er0 = w * P
            rec = sb.tile([P, NH], F32, tag="rec")
            nc.vector.tensor_scalar(rec[:], num3[:, w, NHF:NHF + NH],
                                    ceps[:, 0:1], None, OP.add)
            nc.vector.reciprocal(rec[:], rec[:])
            o1 = sb.tile([P, NHF], F32, tag="o1")
            nc.vector.tensor_tensor(
                o1[:].rearrange("p (h f) -> p h f", f=FOUT),
                num3[:, w, 0:NHF].rearrange("p (h f) -> p h f", f=FOUT),
                rec[:, :, None].to_broadcast([P, NH, FOUT]),
                OP.mult)
            nc.vector.tensor_tensor(o1[:], o1[:], xb3[:, w, :], OP.add)
            nm = sb.tile([P, NHF], F32, tag="nm")
            nc.vector.tensor_scalar_min(nm[:], o1[:], 0.0)
            en = sb.tile([P, NHF], F32, tag="en")
            nc.scalar.activation(en[:], nm[:], ACT.Exp, bias=zcol[:])
            pos1 = sb.tile([P, NHF], F32, tag="pos1")
            nc.vector.tensor_scalar(pos1[:], o1[:], 0.0, -1.0, OP.max, OP.add)
            nc.vector.tensor_tensor(en[:], en[:], pos1[:], OP.add)
            nc.sync.dma_start(out[er0:er0 + P, :], en[:])

    nc.compile()
    return nc


def _make_inputs(x, edge_index, w_mat, a_src, a_trg, bias, n_cores,
                 padskip=False):
    n_nodes = x.shape[0]
    npc = n_nodes // n_cores
    nw = (npc + P - 1) // P
    npad = nw * P
    n_tab = 392 * P  # keep in sync with build_bass
    half = n_tab // 2
    t_a, t_b, gidx, rel_arr, selt = _prepare_edges(edge_index, n_nodes,
                                                   n_cores, half,
                                                   padskip=padskip)
    amat = np.zeros((NHF, 2 * NH), np.float32)
    for h in range(NH):
        amat[h * FOUT:(h + 1) * FOUT, h] = a_src[h]
        amat[h * FOUT:(h + 1) * FOUT, NH + h] = a_trg[h]
    x = np.ascontiguousarray(x, dtype=np.float32)
    w_mat = np.ascontiguousarray(w_mat, dtype=np.float32)
    wcat = np.concatenate([w_mat, w_mat @ amat], axis=1).astype(np.float16)
    xT = np.zeros((FIN, n_tab), np.float16)
    xT[:, 0:n_nodes] = x.T.astype(np.float16)
    bias = np.asarray(bias, dtype=np.float32).reshape(1, NHF)
    in_maps = []
    for c in range(n_cores):
        xloc = x[c * npc:(c + 1) * npc]
        xlocT = np.zeros((FIN, npad), np.float16)
        xlocT[:, 0:npc] = xloc.T.astype(np.float16)
        xbloc = np.zeros((npad, NHF), np.float32)
        xbloc[0:npc] = xloc + bias
        in_maps.append({
            "xT": xT,
            "xlocT": xlocT,
            "xbloc": xbloc,
            "wcat": wcat,
            "gidx": np.ascontiguousarray(gidx[c]),
            "trg_rel": np.ascontiguousarray(rel_arr[c]),
            "selt": np.ascontiguousarray(selt[c]),
        })
    return t_a, t_b, in_maps


def kernel(x, edge_index, W, a_src, a_trg, bias, _trace=False):
    from concourse.bass_utils import run_bass_kernel_spmd

    n_cores = N_CORES
    x = np.asarray(x)
    n_nodes = x.shape[0]
    npc = n_nodes // n_cores
    t_a, t_b, in_maps = _make_inputs(np.asarray(x), np.asarray(edge_index),
                                     np.asarray(W), np.asarray(a_src),
                                     np.asarray(a_trg), np.asarray(bias),
                                     n_cores)
    nc = build_bass(n_nodes, n_cores, t_a, t_b)
    res = run_bass_kernel_spmd(nc, in_maps, core_ids=list(range(n_cores)),
                               trace=_trace)
    out = np.concatenate([res.results[c]["out"][0:npc]
                          for c in range(n_cores)], axis=0)
    if _trace:
        kernel.last_results = res
    return out.astype(np.float32)


# revision 43
# speedup vs baseline: 1.1122x; 1.1122x over previous
# GAT (graph attention) layer on 8 Trainium2 NeuronCores.
#
# Strategy: target-sharded edges. Each core owns 1/8 of the target nodes and
# processes exactly the edges pointing into its range, so the softmax
# denominator and the weighted-feature aggregation are core-local matmuls.
# There is NO collective: the reference's global-max exp shift cancels
# exactly between numerator and denominator (both carry exp(-24) here), and
# its +1e-16 epsilon is replaced by max(denom, 1e-30), which only differs on
# zero-in-degree targets (where both give att=0).
#
# Per core:
#   Phase T1: proj|s_src for ALL nodes from host-transposed xT via one matmul
#     per 128-node tile -> bf16 gather table rows [proj(128)|ss_hi(4)|ss_lo(4)]
#     at 512B stride in DRAM.
#   Phase T2: local s_trg table (hi/lo bf16) into a persistent SBUF tile.
#   Phase E (fused E+F), per window of 128 target nodes: two dma_gathers fetch
#     per-edge source rows; st per edge via one-hot fp8 matmul against the
#     local s_trg window column; z = ss + st, ex = exp(leaky(z) - 24);
#     sel one-hot built on-chip (is_equal vs iota); ONE accumulating matmul
#     per edge tile with lhsT=sel, rhs=[proj*ex | ex] yields
#     [target, features|denoms] directly; divide, add skip(x+bias), ELU, out.
import sys
from contextlib import ExitStack

import numpy as np

sys.path.insert(0, "/opt/trn_rl_repo")

import ml_dtypes  # noqa: E402

import concourse.bass as bass  # noqa: E402,F401
import concourse.bass_isa as bass_isa  # noqa: E402
import concourse.mybir as mybir  # noqa: E402
import concourse.tile as tile  # noqa: E402
from concourse import bacc  # noqa: E402

P = 128
NH, FOUT = 4, 32
NHF = NH * FOUT  # 128
FIN = 128
ROW = 2 * P  # table row: 256 bf16 = 512B (136 used)
LEAKY = 0.2
SHIFT = 24.0
N_CORES = 8
F32 = mybir.dt.float32
F16 = mybir.dt.float16
BF16 = mybir.dt.bfloat16
FP8 = mybir.dt.float8e4
I16 = mybir.dt.int16
I32 = mybir.dt.int32
OP = mybir.AluOpType
ACT = mybir.ActivationFunctionType
BF = ml_dtypes.bfloat16
F8 = ml_dtypes.float8_e4m3
GT = 3  # node tiles per phase-T group (PSUM: 3*136 f32 <= 512)


def _wrap16(flat):
    """[..., L] -> dma_gather layout [..., 16, L//16] replicated to 128 rows."""
    L = flat.shape[-1]
    w = flat.reshape(flat.shape[:-1] + (L // 16, 16))
    w = np.swapaxes(w, -1, -2)  # [..., 16, L//16]
    return np.tile(w, (1, 1, 8, 1)).reshape(flat.shape[:-1] + (P, L // 16))


def _prepare_edges(edge_index, n_nodes, n_cores, half, padskip=False):
    npc = n_nodes // n_cores  # 6250
    nw = (npc + P - 1) // P  # 49
    src = np.ascontiguousarray(edge_index[0]).astype(np.int64)
    trg = np.ascontiguousarray(edge_index[1]).astype(np.int64)
    E = src.shape[0]
    wglob = (trg // npc) * nw + (trg % npc) // P
    isb = (src >= half).astype(np.int64)
    order = np.argsort(wglob * 2 + isb, kind="stable")
    src_s, trg_s, wg_s, isb_s = src[order], trg[order], wglob[order], isb[order]
    nwin = n_cores * nw
    cnt = np.bincount(wg_s * 2 + isb_s, minlength=2 * nwin)
    t_a = max(1, int(np.ceil(cnt[0::2].max() / P)))
    t_b = max(1, int(np.ceil(cnt[1::2].max() / P)))
    t_eff = t_a + t_b
    gkey = wg_s * 2 + isb_s
    gstart = np.concatenate([[0], np.cumsum(np.bincount(gkey, minlength=2 * nwin))])[:-1]
    jj = np.arange(E) - gstart[gkey]
    t_loc = jj // P
    p_idx = jj % P
    t_idx = np.where(isb_s == 1, t_a + t_loc, t_loc)
    c = wg_s // nw
    wloc = wg_s % nw
    rel = (trg_s % npc) - wloc * P  # 0..127

    fill = -1 if padskip else 0  # -1: trailing pads are skipped by dma_gather
    idx_a = np.full((n_cores, nw, t_a * P), fill, np.int16)
    idx_b = np.full((n_cores, nw, t_b * P), fill, np.int16)
    ma = isb_s == 0
    idx_a[c[ma], wloc[ma], t_loc[ma] * P + p_idx[ma]] = src_s[ma].astype(np.int16)
    mb = isb_s == 1
    idx_b[c[mb], wloc[mb], t_loc[mb] * P + p_idx[mb]] = (src_s[mb] - half).astype(np.int16)

    rel_arr = np.full((n_cores, nw * P, t_eff), -1.0, np.float32)
    rel_arr[c, wloc * P + p_idx, t_idx] = rel
    selt = np.zeros((n_cores, nw * P, t_eff * P), F8)
    selt[c, wloc * P + rel, t_idx * P + p_idx] = 1.0

    ia = _wrap16(idx_a)  # [nc, nw, 128, t_a*8]
    ib = _wrap16(idx_b)
    gidx = np.concatenate([ia, ib], axis=-1).reshape(n_cores, nw * P, t_eff * 8)
    return t_a, t_b, gidx, rel_arr.astype(BF), selt


def build_bass(n_nodes, n_cores, t_a, t_b, mock_cc=False, phases="full",
               sp=False, qb=0, tail="pe", ngather=False, nselt=False,
               padskip=False):
    npc = n_nodes // n_cores  # 6250
    nw = (npc + P - 1) // P  # 49
    npad = nw * P  # 6272
    ntp = 392  # node tiles incl padding (130 groups of 3 + 1 group of 2)
    n_tab = ntp * P  # 50176
    half = n_tab // 2  # 25088
    t_eff = t_a + t_b
    nc = bacc.Bacc("TRN2", target_bir_lowering=False, debug=False,
                   num_devices=n_cores)

    xT = nc.dram_tensor("xT", [FIN, n_tab], F16, kind="ExternalInput")
    xlocT = nc.dram_tensor("xlocT", [FIN, npad], F16, kind="ExternalInput")
    wcat_in = nc.dram_tensor("wcat", [FIN, NHF + 2 * NH], F16,
                             kind="ExternalInput")
    if phases != "T":
        gidx = nc.dram_tensor("gidx", [nw * P, t_eff * 8], I16,
                              kind="ExternalInput")
        trg_rel = nc.dram_tensor("trg_rel", [nw * P, t_eff], BF16,
                                 kind="ExternalInput")
        selt_in = nc.dram_tensor("selt", [nw * P, t_eff * P], FP8,
                                 kind="ExternalInput")
    if phases == "full":
        xbloc = nc.dram_tensor("xbloc", [npad, NHF], F32, kind="ExternalInput")
    out = nc.dram_tensor("out", [npad, NHF], F32, kind="ExternalOutput")

    tab = nc.dram_tensor("tab", [n_tab, ROW], BF16)

    with tile.TileContext(nc) as tc, ExitStack() as ctx:
        const = ctx.enter_context(tc.tile_pool(name="const", bufs=1))

        c_i32 = const.tile([P, P], I32)
        nc.gpsimd.iota(c_i32[:], pattern=[[1, P]], base=0, channel_multiplier=0)
        c_bf = const.tile([P, P], BF16)
        nc.vector.tensor_copy(c_bf[:], c_i32[:])

        wcat = const.tile([FIN, NHF + 2 * NH], F16)
        nc.sync.dma_start(wcat[:], wcat_in[:])
        neg24 = const.tile([P, 1], F32)
        nc.gpsimd.memset(neg24[:], -SHIFT)
        zcol = const.tile([P, 1], F32)
        nc.gpsimd.memset(zcol[:], 0.0)
        tabc = const.tile([P, nw * 2 * NH], BF16)  # local s_trg hi/lo
        tabc3 = tabc[:].rearrange("p (w c) -> p w c", c=2 * NH)
        zlmax = const.tile([P, (t_a + t_b) * NH], F32)
        nc.gpsimd.memset(zlmax[:], -1e30)
        negbig = const.tile([P, 1], F32)
        nc.gpsimd.memset(negbig[:], -1e30)
        zerob = const.tile([P, 1], BF16)
        nc.gpsimd.memset(zerob[:], 0.0)
        num_sb = const.tile([P, nw * (NHF + NH)], F32)  # per-window [feat|den]
        num3 = num_sb[:].rearrange("p (w c) -> p w c", c=NHF + NH)

        # one-shot preloads of all per-window metadata (SBUF-resident)
        if phases != "T":
            gidx_sb = const.tile([P, nw * t_eff * 8], I16)
            gidx3 = gidx_sb[:].rearrange("p (w c) -> p w c", c=t_eff * 8)
            nc.sync.dma_start(
                gidx3, gidx[:].rearrange("(w p) c -> p w c", p=P))
            rel_sb = const.tile([P, nw * t_eff], BF16)
            rel3 = rel_sb[:].rearrange("p (w c) -> p w c", c=t_eff)
            nc.sync.dma_start(
                rel3, trg_rel[:].rearrange("(w p) c -> p w c", p=P))
        if phases == "full":
            xb_sb = const.tile([P, nw * NHF], F32)
            xb3 = xb_sb[:].rearrange("p (w c) -> p w c", c=NHF)
            nc.sync.dma_start(
                xb3, xbloc[:].rearrange("(w p) c -> p w c", p=P))

        # --- phase T1: global gather table [proj | ss_hi | ss_lo] ---
        with tc.tile_pool(name="sbT", bufs=3) as sbT, \
             tc.tile_pool(name="psT", bufs=2, space="PSUM") as psT:
            g0 = 0
            gi = 0
            while g0 < ntp:
                tg = min(GT, ntp - g0)
                cols = tg * P
                xt = sbT.tile([P, GT * P], F16, tag="xt")
                nc.sync.dma_start(xt[:, 0:cols], xT[:, g0 * P:g0 * P + cols])
                ps = psT.tile([P, GT * (NHF + NH)], F32, tag="ps")
                ps3 = ps[:].rearrange("p (t c) -> p t c", c=NHF + NH)
                for t in range(tg):
                    nc.tensor.matmul(ps3[:, t, :],
                                     lhsT=xt[:, t * P:(t + 1) * P],
                                     rhs=wcat[:, 0:NHF + NH],
                                     start=True, stop=True)
                tabt = sbT.tile([P, GT * ROW], BF16, tag="tabt")
                t3 = tabt[:].rearrange("p (t c) -> p t c", c=ROW)
                t4 = tabt[:].rearrange("p (t d c) -> p t d c", d=2, c=P)
                # proj into cols 0:128 AND (stale copy) 128:256 so the full
                # 512B row is initialized; ss_hi then overwrites 128:132
                nc.scalar.activation(
                    t4[:, 0:tg, :, :],
                    ps3[:, 0:tg, None, 0:NHF].to_broadcast([P, tg, 2, NHF]),
                    ACT.Copy)
                nc.scalar.activation(t3[:, 0:tg, NHF:NHF + NH],
                                     ps3[:, 0:tg, NHF:NHF + NH], ACT.Copy)
                nc.vector.tensor_tensor(t3[:, 0:tg, NHF + NH:NHF + 2 * NH],
                                        ps3[:, 0:tg, NHF:NHF + NH],
                                        t3[:, 0:tg, NHF:NHF + NH], OP.subtract)
                od = tab[g0 * P:g0 * P + cols, :]
                od3 = od.rearrange("(t p) c -> p t c", p=P)
                nc.sync.dma_start(od3, t3[:, 0:tg, :])
                g0 += tg

            # --- phase T2: local s_trg hi/lo into SBUF (49 tiles) ---
            g0 = 0
            while g0 < nw:
                tg = min(GT, nw - g0)
                cols = tg * P
                xt = sbT.tile([P, GT * P], F16, tag="xt")
                nc.sync.dma_start(xt[:, 0:cols], xlocT[:, g0 * P:g0 * P + cols])
                ps = psT.tile([P, GT * (NHF + NH)], F32, tag="ps")
                ps3 = ps[:].rearrange("p (t c) -> p t c", c=NHF + NH)
                for t in range(tg):
                    nc.tensor.matmul(ps3[:, t, 0:NH],
                                     lhsT=xt[:, t * P:(t + 1) * P],
                                     rhs=wcat[:, NHF + NH:NHF + 2 * NH],
                                     start=True, stop=True)
                nc.scalar.activation(tabc3[:, g0:g0 + tg, 0:NH],
                                     ps3[:, 0:tg, 0:NH], ACT.Copy)
                nc.vector.tensor_tensor(tabc3[:, g0:g0 + tg, NH:2 * NH],
                                        ps3[:, 0:tg, 0:NH],
                                        tabc3[:, g0:g0 + tg, 0:NH], OP.subtract)
                g0 += tg

        # --- phase E: per target window, fused edge + output stage ---
        sb = ctx.enter_context(tc.tile_pool(name="sb", bufs=3))
        sbg = ctx.enter_context(tc.tile_pool(name="sbg", bufs=3))
        psE = ctx.enter_context(tc.tile_pool(name="psE", bufs=3, space="PSUM"))

        if phases == "T":  # timing probe: stop after table build
            dummy = sb.tile([P, NHF], F32, tag="dummy")
            nc.vector.tensor_copy(dummy[:], wcat[:, 0:NHF])
            nc.sync.dma_start(out[0:P, :], dummy[:])

        for w in range(nw if phases != "T" else 0):
            er0 = w * P
            selt = sbg.tile([P, t_eff * P], FP8, tag="selt")
            if nselt:  # timing probe: selt DMA replaced by DVE memset
                nc.vector.memset(selt[:], 1.0)
            else:
                nc.sync.dma_start(selt[:], selt_in[er0:er0 + P, :])

            gath = sbg.tile([P, t_eff * ROW], BF16, tag="gath")
            g3 = gath[:].rearrange("p (t c) -> p t c", c=ROW)
            if ngather:  # timing probe: gathers replaced by DVE memset
                nc.vector.memset(gath[:], 0.0)
            else:
                nc.gpsimd.dma_gather(
                    out_ap=g3[:, 0:t_a, :], in_ap=tab[0:half, :],
                    idxs_ap=gidx3[:, w, 0:t_a * 8], num_idxs=t_a * P,
                    num_idxs_reg=t_a * P, elem_size=ROW, single_packet=sp)
                nc.gpsimd.dma_gather(
                    out_ap=g3[:, t_a:t_eff, :], in_ap=tab[half:n_tab, :],
                    idxs_ap=gidx3[:, w, t_a * 8:t_eff * 8], num_idxs=t_b * P,
                    num_idxs_reg=t_b * P, elem_size=ROW, single_packet=sp,
                    queue_num=qb)

            # st per edge: one-hot fp8 select matmul against local window col
            ps_st = psE.tile([P, t_eff * 2 * NH], F32, tag="ps_st")
            st3 = ps_st[:].rearrange("p (t h) -> p t h", h=2 * NH)
            for t in range(t_eff):
                nc.tensor.matmul(st3[:, t, :],
                                 lhsT=selt[:, t * P:(t + 1) * P],
                                 rhs=tabc3[:, w, :], start=True, stop=True)

            # z = ss_hi + ss_lo + st_hi + st_lo ; ex = exp(leaky(z) - 24)
            z = sb.tile([P, t_eff * NH], F32, tag="z")
            z3 = z[:].rearrange("p (t h) -> p t h", h=NH)
            nc.vector.tensor_tensor(z3, g3[:, :, NHF:NHF + NH],
                                    g3[:, :, NHF + NH:NHF + 2 * NH], OP.add)
            nc.vector.tensor_tensor(z3, z3, st3[:, :, 0:NH], OP.add)
            nc.vector.tensor_tensor(z3, z3, st3[:, :, NH:2 * NH], OP.add)
            zl = sb.tile([P, t_eff * NH], F32, tag="zl")
            nc.vector.tensor_scalar_mul(zl[:], z[:], LEAKY)
            nc.vector.tensor_tensor(zl[:], zl[:], z[:], OP.max)
            if padskip:
                # pad slots hold stale gather data; force them to -1e30 so
                # neither zlmax nor ex sees garbage
                padmask = sb.tile([P, t_eff], BF16, tag="padmask")
                nc.vector.tensor_scalar(padmask[:], rel3[:, w, :], -1.0, None,
                                        OP.is_equal)
                zl3 = zl[:].rearrange("p (t h) -> p t h", h=NH)
                nc.vector.copy_predicated(
                    zl3, padmask[:, :, None].to_broadcast([P, t_eff, NH]),
                    negbig[:, 0:1, None].to_broadcast([P, t_eff, NH]))
            nc.vector.tensor_tensor(zlmax[:], zlmax[:], zl[:], OP.max)
            ex = sb.tile([P, t_eff * NH], BF16, tag="ex")
            nc.scalar.activation(ex[:], zl[:], ACT.Exp, bias=neg24[:])
            ex3 = ex[:].rearrange("p (t h) -> p t h", h=NH)

            # sel one-hot (edge-partition layout), built on-chip
            sel = sbg.tile([P, t_eff * P], BF16, tag="sel")
            relw = rel3[:, w, :]
            nc.vector.tensor_tensor(
                sel[:].rearrange("p (t q) -> p t q", q=P),
                relw[:, :, None].to_broadcast([P, t_eff, P]),
                c_bf[:, None, :].to_broadcast([P, t_eff, P]),
                OP.is_equal)

            # wgt = [proj * ex | ex]
            wgt = sbg.tile([P, t_eff * (NHF + NH)], BF16, tag="wgt")
            w3 = wgt[:].rearrange("p (t c) -> p t c", c=NHF + NH)
            nc.vector.tensor_tensor(
                w3[:, :, 0:NHF].rearrange("p t (h f) -> p t h f", f=FOUT),
                g3[:, :, 0:NHF].rearrange("p t (h f) -> p t h f", f=FOUT),
                ex3[:, :, :, None].to_broadcast([P, t_eff, NH, FOUT]),
                OP.mult)
            nc.vector.tensor_copy(w3[:, :, NHF:NHF + NH], ex3)
            if padskip:
                # stale-garbage proj x ex can be NaN; force pad slots to 0
                nc.vector.copy_predicated(
                    w3, padmask[:, :, None].to_broadcast([P, t_eff, NHF + NH]),
                    zerob[:, 0:1, None].to_broadcast([P, t_eff, NHF + NH]))

            # accumulate [targets, feats | denom] in one PSUM tile
            ps_o = psE.tile([P, NHF + NH], F32, tag="ps_o")
            for t in range(t_eff):
                nc.tensor.matmul(ps_o[:], lhsT=sel[:, t * P:(t + 1) * P],
                                 rhs=wgt[:, t * (NHF + NH):(t + 1) * (NHF + NH)],
                                 start=(t == 0), stop=(t == t_eff - 1))
            nc.vector.tensor_copy(num3[:, w, :], ps_o[:])

        if phases in ("TE", "TEC"):  # timing probes
            o0 = sb.tile([P, NHF], F32, tag="o0")
            nc.vector.tensor_copy(o0[:], num3[:, 0, 0:NHF])
            nc.sync.dma_start(out[0:P, :], o0[:])

        if phases in ("full", "TEC"):
            # --- global max M (tiny AllReduce) -> ceps = 1e-16*exp(M-24) ---
            zm1 = sb.tile([P, 1], F32, tag="zm1")
            nc.vector.tensor_reduce(zm1[:], zlmax[:],
                                    axis=mybir.AxisListType.X, op=OP.max)
            dram = ctx.enter_context(tc.tile_pool(name="dram", bufs=1,
                                                  space="DRAM"))
            psC = ctx.enter_context(tc.tile_pool(name="psC", bufs=1,
                                                 space="PSUM"))
            if tail == "pe":
                # cross-partition max without gpsimd: PE transpose + DVE
                from concourse.masks import make_identity
                ident = const.tile([P, P], F32)
                make_identity(nc, ident[:])
                ps_zr = psC.tile([1, P], F32, tag="ps_zr")
                nc.tensor.matmul(ps_zr[:], lhsT=zm1[:], rhs=ident[:],
                                 start=True, stop=True)
                zrow = sb.tile([1, P], F32, tag="zrow")
                nc.vector.tensor_copy(zrow[:], ps_zr[:])
                pm = sb.tile([1, 1], F32, tag="pm1")
                nc.vector.tensor_reduce(pm[:], zrow[:],
                                        axis=mybir.AxisListType.X, op=OP.max)
                cc_in = dram.tile([1, 1], F32)
                cc_out = dram.tile([1, 1], F32)
                nc.sync.dma_start(cc_in[:], pm[:])
                if mock_cc:
                    mglob = nc.dram_tensor("mglob", [1, 1], F32,
                                           kind="ExternalInput")
                    nc.sync.dma_start(cc_out[:], mglob[:])
                else:
                    nc.gpsimd.collective_compute(
                        "AllReduce", OP.max,
                        replica_groups=[list(range(n_cores))],
                        ins=[cc_in.opt()], outs=[cc_out.opt()])
                cs = sb.tile([1, 1], F32, tag="cs")
                nc.sync.dma_start(cs[:], cc_out[:])
                nc.scalar.activation(cs[:], cs[:], ACT.Exp, bias=neg24[:1])
                nc.vector.tensor_scalar_mul(cs[:], cs[:], 1e-16)
                ones_col = const.tile([1, P], F32)
                nc.gpsimd.memset(ones_col[:], 1.0)
                ps_cb = psC.tile([P, 1], F32, tag="ps_cb")
                nc.tensor.matmul(ps_cb[:], lhsT=ones_col[:], rhs=cs[:],
                                 start=True, stop=True)
                ceps = const.tile([P, 1], F32)
                nc.vector.tensor_copy(ceps[:], ps_cb[:])
            else:
                pm = sb.tile([P, 1], F32, tag="pm")
                nc.gpsimd.partition_all_reduce(pm[:], zm1[:], channels=P,
                                               reduce_op=bass_isa.ReduceOp.max)
                cc_in = dram.tile([P, 1], F32)
                cc_out = dram.tile([P, 1], F32)
                nc.sync.dma_start(cc_in[:], pm[:])
                if mock_cc:
                    mglob = nc.dram_tensor("mglob", [P, 1], F32,
                                           kind="ExternalInput")
                    nc.sync.dma_start(cc_out[:], mglob[:])
                else:
                    nc.gpsimd.collective_compute(
                        "AllReduce", OP.max,
                        replica_groups=[list(range(n_cores))],
                        ins=[cc_in.opt()], outs=[cc_out.opt()])
                ceps = const.tile([P, 1], F32)
                nc.sync.dma_start(ceps[:], cc_out[:])
                nc.scalar.activation(ceps[:], ceps[:], ACT.Exp, bias=neg24[:])
                nc.vector.tensor_scalar_mul(ceps[:], ceps[:], 1e-16)

        # --- phase F2: divide (+eps), skip+bias, ELU ---
        for w in range(nw if phases == "full" else 0):
            er0 = w * P
            rec = sb.tile([P, NH], F32, tag="rec")
            nc.vector.tensor_scalar(rec[:], num3[:, w, NHF:NHF + NH],
                                    ceps[:, 0:1], None, OP.add)
            nc.vector.reciprocal(rec[:], rec[:])
            o1 = sb.tile([P, NHF], F32, tag="o1")
            nc.vector.tensor_tensor(
                o1[:].rearrange("p (h f) -> p h f", f=FOUT),
                num3[:, w, 0:NHF].rearrange("p (h f) -> p h f", f=FOUT),
                rec[:, :, None].to_broadcast([P, NH, FOUT]),
                OP.mult)
            nc.vector.tensor_tensor(o1[:], o1[:], xb3[:, w, :], OP.add)
            nm = sb.tile([P, NHF], F32, tag="nm")
            nc.vector.tensor_scalar_min(nm[:], o1[:], 0.0)
            en = sb.tile([P, NHF], F32, tag="en")
            nc.scalar.activation(en[:], nm[:], ACT.Exp, bias=zcol[:])
            pos1 = sb.tile([P, NHF], F32, tag="pos1")
            nc.vector.tensor_scalar(pos1[:], o1[:], 0.0, -1.0, OP.max, OP.add)
            nc.vector.tensor_tensor(en[:], en[:], pos1[:], OP.add)
            nc.sync.dma_start(out[er0:er0 + P, :], en[:])

    nc.compile()
    return nc


def _make_inputs(x, edge_index, w_mat, a_src, a_trg, bias, n_cores,
                 padskip=False):
    n_nodes = x.shape[0]
    npc = n_nodes // n_cores
    nw = (npc + P - 1) // P
    npad = nw * P
    n_tab = 392 * P  # keep in sync with build_bass
    half = n_tab // 2
    t_a, t_b, gidx, rel_arr, selt = _prepare_edges(edge_index, n_nodes,
                                                   n_cores, half,
                                                   padskip=padskip)
    amat = np.zeros((NHF, 2 * NH), np.float32)
    for h in range(NH):
        amat[h * FOUT:(h + 1) * FOUT, h] = a_src[h]
        amat[h * FOUT:(h + 1) * FOUT, NH + h] = a_trg[h]
    x = np.ascontiguousarray(x, dtype=np.float32)
    w_mat = np.ascontiguousarray(w_mat, dtype=np.float32)
    wcat = np.concatenate([w_mat, w_mat @ amat], axis=1).astype(np.float16)
    xT = np.zeros((FIN, n_tab), np.float16)
    xT[:, 0:n_nodes] = x.T.astype(np.float16)
    bias = np.asarray(bias, dtype=np.float32).reshape(1, NHF)
    in_maps = []
    for c in range(n_cores):
        xloc = x[c * npc:(c + 1) * npc]
        xlocT = np.zeros((FIN, npad), np.float16)
        xlocT[:, 0:npc] = xloc.T.astype(np.float16)
        xbloc = np.zeros((npad, NHF), np.float32)
        xbloc[0:npc] = xloc + bias
        in_maps.append({
            "xT": xT,
            "xlocT": xlocT,
            "xbloc": xbloc,
            "wcat": wcat,
            "gidx": np.ascontiguousarray(gidx[c]),
            "trg_rel": np.ascontiguousarray(rel_arr[c]),
            "selt": np.ascontiguousarray(selt[c]),
        })
    return t_a, t_b, in_maps


def kernel(x, edge_index, W, a_src, a_trg, bias, _trace=False):
    from concourse.bass_utils import run_bass_kernel_spmd

    n_cores = N_CORES
    x = np.asarray(x)
    n_nodes = x.shape[0]
    npc = n_nodes // n_cores
    t_a, t_b, in_maps = _make_inputs(np.asarray(x), np.asarray(edge_index),
                                     np.asarray(W), np.asarray(a_src),
                                     np.asarray(a_trg), np.asarray(bias),
                                     n_cores)
    nc = build_bass(n_nodes, n_cores, t_a, t_b)
    res = run_bass_kernel_spmd(nc, in_maps, core_ids=list(range(n_cores)),
                               trace=_trace)
    out = np.concatenate([res.results[c]["out"][0:npc]
                          for c in range(n_cores)], axis=0)
    if _trace:
        kernel.last_results = res
    return out.astype(np.float32)
